# revision 1
# baseline (speedup 1.0000x reference)
"""Trainium2 Bass kernel for causal self-attention with RoPE.

Problem: B=1, S=2048, D=4096, H=32 heads, head_dim=128, fp32.
Sharding: tensor-parallel over heads across 8 NeuronCores — each core owns 4
heads (column-shard of Wq/Wk/Wv, row-shard of Wo) and produces a partial
[S, D] output; the host sums the 8 partials (the "all-reduce").

Per-core pipeline (v2), phase order A-q1, B, A-q2..4, C/D:
  B: V = hidden @ Wv^T (bf16); hidden tiles stream on the SP DMA queue,
     all weight/trig/mask DMAs ride the Pool-engine DMA queue.
  A: Q^T, K^T = W_h @ hidden^T per head (bf16, fp32 PSUM). PSUM banks are
     released via fast bf16 staging copies (split ACT/DVE); RoPE runs
     deferred from SBUF in bf16 (2x DVE rate); the rotate-half layout is
     made by two SBUF->SBUF DMAs (engines cannot cross partitions SB->SB);
     the last quarter's RoPE is deferred into phase C.
  C: per q-chunk of 512, per head: scores^T = K^T.T @ Q^T chunk, causal
     block-skip plus column-narrowed diagonal tiles, exp (no max-sub, bf16).
     The softmax denominator accumulates on DVE/GpSimd into two bf16
     accumulators (t=0/1 exp writes them directly), finished by one cheap
     PE ones-matmul pair per (head, chunk) — replacing 10 per-tile
     denominator matmuls with 2 — then reciprocal (DVE) and a GpSimd
     partition_broadcast; the finish is delayed one head so PE never waits
     on the accumulate chains.
  D: output projection for chunk j is emitted after attention chunk j+1
     (one-chunk software pipeline) so it never stalls on the softmax tail.
"""
import math
import sys

import numpy as np

sys.path.insert(0, "/opt/trn_rl_repo")

import ml_dtypes

import concourse.bass as bass
import concourse.tile as tile
from concourse import bacc
from concourse import bass_isa
from concourse import mybir
from concourse.bass_utils import run_bass_kernel_spmd

F32 = mybir.dt.float32
F32R = mybir.dt.float32r
BF16 = mybir.dt.bfloat16
EXP = mybir.ActivationFunctionType.Exp
RADD = bass_isa.ReduceOp.add

S, D = 2048, 4096
HL = 4            # local heads per core
HD = 128
NJ, CH = 4, 512   # q-chunks
NK = 32           # d-tiles of 128 (contraction for projections)
NCORES = 8


def _r(ap):
    return ap.bitcast(F32R)


def build_nc(reps=1):
    nc = bacc.Bacc("TRN2", target_bir_lowering=False, debug=False,
                   num_devices=NCORES)

    aps = {}
    for nm, shape, dt in (
            ("hT", [D, S], BF16), ("wqT", [D, 512], BF16),
            ("wkT", [D, 512], BF16), ("wvT", [D, 512], BF16),
            ("woT", [512, D], BF16),
            ("cosT", [HD, S], BF16), ("sinTm", [HD, S], BF16),
            ("maskT", [4, HD, CH], BF16)):
        aps[nm] = nc.dram_tensor(nm, shape, dt, kind="ExternalInput").ap()
    out = nc.dram_tensor("out", [S, D], BF16, kind="ExternalOutput").ap()

    with tile.TileContext(nc) as tc, \
         nc.allow_low_precision(reason="bf16 staging/trig/ex tiles; "
                                "accumulation stays fp32 in PSUM/SBUF"):
        for _ in range(reps):
            build_body(tc, aps, out)
    nc.compile()
    return nc


def build_body(tc, aps, out):
    nc = tc.nc
    hT = aps["hT"]

    small = tc.alloc_tile_pool(name="small", bufs=1)
    mk = [small.tile([HD, CH], BF16, tag=f"mk{t}", name=f"mk{t}")
          for t in range(4)]
    ones = small.tile([128, 1], BF16, tag="ones", name="ones")
    nc.vector.memset(ones[:], 1.0)
    ones_row = small.tile([1, 128], BF16, tag="ones_row", name="ones_row")
    nc.vector.memset(ones_row[:], 1.0)

    # long-lived tensors on the right SBUF stack
    pot = tc.alloc_tile_pool(name="pot", bufs=1, side="right")
    OT = [pot.tile([HD, S], BF16, tag=f"oT{h}", name=f"oT{h}")
          for h in range(HL)]

    pqk = tc.alloc_tile_pool(name="pqk", bufs=1)
    qT = [pqk.tile([HD, S], BF16, tag=f"qT{h}", name=f"qT{h}")
          for h in range(HL)]
    kTt = [pqk.tile([HD, S], BF16, tag=f"kT{h}", name=f"kT{h}")
           for h in range(HL)]

    pv_pool = tc.alloc_tile_pool(name="pv_pool", bufs=1, side="right")
    V = [pv_pool.tile([128, 512], BF16, tag=f"v{i}", name=f"v{i}")
         for i in range(16)]

    # staging ring shared by A's PSUM drains and C's exp tiles; softmax
    # split accumulators (even-t on DVE, odd-t on GpSimd). trigA/ropeA are
    # body-lived so the last quarter's RoPE can be deferred into phase C.
    stageP = tc.alloc_tile_pool(name="stageP", bufs=16)
    stRotP = tc.alloc_tile_pool(name="stRotP", bufs=6)
    accA = tc.alloc_tile_pool(name="accA", bufs=2)
    accB = tc.alloc_tile_pool(name="accB", bufs=2)
    miscC = tc.alloc_tile_pool(name="miscC", bufs=2)
    trigA = tc.alloc_tile_pool(name="trigA", bufs=1)
    ropeA = tc.alloc_tile_pool(name="ropeA", bufs=1)

    # ---- phases A (Q^T/K^T quarters) and B (V), order: A-q1, B, A-q2..4.
    # Starting with an A quarter keeps the opening DMA light (weights
    # trickle at k-loop rate) while Wv's 4MB bulk loads in the background.
    with tc.tile_pool(name="wA", bufs=5) as wA, \
         tc.tile_pool(name="htS", bufs=5) as htS:
        trig = {}
        wqk = {}

        def emit_trig():
            for nm, srcn in (("cq", "cosT"), ("sq", "sinTm")):
                t = trigA.tile([HD, S], BF16, tag=nm, name=f"trig_{nm}")
                nc.gpsimd.dma_start(t[:], aps[srcn])
                trig[nm] = t

        def emit_prefetch(hp):
            # A-phase weights on the Pool DMA queue (8-deep rings: no
            # ring-slot waits, never blocks the hidden-tile stream).
            # The opening tiles are split so the first matmul's weights
            # land in ~128KB instead of ~512KB.
            for kg in range(NK // 8):
                for (tg, srcn) in (("wq", "wqT"), ("wk", "wkT")):
                    wt = wA.tile([128, 8 * 256], BF16, tag=tg,
                                 name=f"{tg}{hp}_{kg}")
                    splits = ((0, 2), (2, 8)) if hp == 0 and kg == 0                         else ((0, 8),)
                    for (a, b) in splits:
                        nc.gpsimd.dma_start(
                            wt[:, a*256:b*256].rearrange(
                                "p (k f) -> p k f", k=b-a),
                            aps[srcn][kg*1024+a*128:kg*1024+b*128,
                                      hp*256:(hp+1)*256].rearrange(
                                "(k p) f -> p k f", p=128))
                    wqk[(tg, hp, kg)] = wt

        WV_AT = (10, 14, 18, 22, 26)

        def emit_quarter(hp, jp, psA, defer_rope=False):
            pq, pk = {}, {}
            for hh in range(2):
                for jj in range(2):
                    pq[(hh, jj)] = psA.tile(
                        [128, CH], F32, tag=f"pq{hh}{jj}",
                        name=f"pq{hp}{jp}{hh}{jj}")
                    pk[(hh, jj)] = psA.tile(
                        [128, CH], F32, tag=f"pk{hh}{jj}",
                        name=f"pk{hp}{jp}{hh}{jj}")
            for k in range(NK):
                htf = htS.tile([128, 2 * CH], BF16, tag="htf", bufs=7,
                               name=f"htA{hp}{jp}{k}")
                nc.sync.dma_start(
                    htf[:], hT[k*128:(k+1)*128, jp*1024:(jp+1)*1024])
                for hh in range(2):
                    ko = (k % 8) * 256 + hh * 128
                    wq_t = wqk[("wq", hp, k // 8)][:, ko:ko+128]
                    wk_t = wqk[("wk", hp, k // 8)][:, ko:ko+128]
                    for jj in range(2):
                        rhs = htf[:, jj*CH:(jj+1)*CH]
                        nc.tensor.matmul(
                            pq[(hh, jj)][:], wq_t, rhs,
                            start=(k == 0), stop=(k == NK - 1))
                        nc.tensor.matmul(
                            pk[(hh, jj)][:], wk_t, rhs,
                            start=(k == 0), stop=(k == NK - 1))
                if hp == 0 and jp == 0 and k in WV_AT:
                    emit_wv(WV_AT.index(k))
            # fast PSUM release: bf16 staging copies (ACT/DVE split).
            # The rotate-half layout is produced by two SBUF->SBUF DMAs on
            # the idle Pool queue (engines cannot cross partitions SB->SB).
            stg = []
            idx = 0
            for hh in range(2):
                for jj in range(2):
                    for ps in (pq[(hh, jj)], pk[(hh, jj)]):
                        st = stageP.tile([128, CH], BF16, tag="stg",
                                         name=f"st{hp}{jp}{idx}")
                        if idx % 2 == 0:
                            nc.scalar.copy(st[:], ps[:])
                        else:
                            nc.vector.tensor_copy(st[:], ps[:])
                        rot = stRotP.tile([128, CH], BF16, tag="rot",
                                          name=f"rot{hp}{jp}{idx}")
                        nc.gpsimd.dma_start(rot[0:64, :], st[64:128, :])
                        nc.gpsimd.dma_start(rot[64:128, :], st[0:64, :])
                        stg.append((st, rot))
                        idx += 1
            # RoPE from SBUF, all-bf16 on DVE (2x rate); optionally deferred
            # into phase C (the data is not read until chunk 2)
            rope_thunks = []
            idx = 0
            for hh in range(2):
                for jj in range(2):
                    h = hp*2 + hh
                    j = jp*2 + jj
                    cj = trig["cq"][:, j*CH:(j+1)*CH]
                    sj = trig["sq"][:, j*CH:(j+1)*CH]
                    for dst in (qT[h], kTt[h]):
                        st, rot = stg[idx]
                        idx += 1

                        def rope(st=st, rot=rot, dst=dst, h=h, j=j, cj=cj,
                                 sj=sj, idx=idx):
                            tcos = ropeA.tile([128, CH], BF16, tag="tcos",
                                              name=f"tc{h}{j}{idx}")
                            nc.vector.tensor_mul(tcos[:], st[:], cj)
                            tsin = ropeA.tile([128, CH], BF16, tag="tsin",
                                              name=f"ts{h}{j}{idx}")
                            nc.vector.tensor_mul(tsin[:], rot[:], sj)
                            nc.vector.tensor_add(
                                dst[:, j*CH:(j+1)*CH], tcos[:], tsin[:])
                        if defer_rope:
                            rope_thunks.append(rope)
                        else:
                            rope()
            return rope_thunks

        emit_prefetch(0)
        emit_trig()
        with tc.tile_pool(name="wvP", bufs=1) as wvP:
            # Wv + masks load mid-quarter-1 on the Pool queue (keeps the
            # opening DMA window clear for the hidden/weight streams)
            wv_sb = wvP.tile([128, NK * 512], BF16, tag="wv", name="wv_sb")

            WV_CH = (4, 4, 8, 8, 8)

            def emit_wv(ci):
                kc0 = sum(WV_CH[:ci])
                nk_c = WV_CH[ci]
                nc.gpsimd.dma_start(
                    wv_sb[:, kc0*512:(kc0+nk_c)*512].rearrange(
                        "p (k f) -> p k f", k=nk_c),
                    aps["wvT"][kc0*128:(kc0+nk_c)*128, :].rearrange(
                        "(k p) f -> p k f", p=128))
                if ci == len(WV_CH) - 1:
                    for t in range(4):
                        nc.gpsimd.dma_start(mk[t][:], aps["maskT"][t])

            with tc.tile_pool(name="psA1", bufs=1, space="PSUM") as psA1:
                emit_quarter(0, 0, psA1)

            with tc.tile_pool(name="psB", bufs=2, space="PSUM") as psB:
                for j in range(NJ):
                    pv = [psB.tile([128, 512], F32, tag=f"pv{i}",
                                   name=f"pv{j}_{i}") for i in range(4)]
                    for k in range(NK):
                        ht = htS.tile([128, CH], BF16, tag="htb",
                                      name=f"htB{j}{k}")
                        nc.sync.dma_start(
                            ht[:], hT[k*128:(k+1)*128, j*CH:(j+1)*CH])
                        for i in range(4):
                            nc.tensor.matmul(
                                pv[i][:], ht[:, i*128:(i+1)*128],
                                wv_sb[:, k*512:(k+1)*512],
                                start=(k == 0), stop=(k == NK - 1))
                    for i in range(4):
                        nc.scalar.copy(V[j*4 + i][:], pv[i][:])

        emit_prefetch(1)
        with tc.tile_pool(name="psA2", bufs=1, space="PSUM") as psA2:
            emit_quarter(0, 1, psA2)
            emit_quarter(1, 0, psA2)
            rope_q4 = emit_quarter(1, 1, psA2, defer_rope=True)

    # --------- phase C (attention) + D (projection), D lags one chunk ----
    with tc.tile_pool(name="wD", bufs=1) as wD, \
         tc.tile_pool(name="outD", bufs=4) as outD, \
         tc.tile_pool(name="psS", bufs=3, space="PSUM") as psS, \
         tc.tile_pool(name="psO", bufs=2, space="PSUM") as psO, \
         tc.tile_pool(name="psD", bufs=1, space="PSUM") as psD, \
         tc.tile_pool(name="psF", bufs=2, space="PSUM") as psF:
        wo_sb = wD.tile([128, HL * D], BF16, tag="wo", name="wo_sb")
        nc.gpsimd.dma_start(
            wo_sb.rearrange("p (h n) -> p h n", h=HL),
            aps["woT"].rearrange("(h p) n -> p h n", p=128))

        pending = []

        def finish_one():
            # softmax finish, delayed one head so the PE denominator
            # matmuls never wait on the accumulate chains
            h0, j0, po0, aa, ab = pending.pop(0)
            pd = psD.tile([1, CH], F32, tag="pd", name=f"pd{h0}_{j0}")
            nc.tensor.matmul(pd[:], ones[:], aa[:],
                             start=True, stop=False)
            nc.tensor.matmul(pd[:], ones[:], ab[:],
                             start=False, stop=True)
            rec = miscC.tile([1, CH], BF16, tag="rec",
                             name=f"rec{h0}_{j0}")
            nc.vector.reciprocal(rec[:], pd[:])
            rb = miscC.tile([128, CH], BF16, tag="rb",
                            name=f"rb{h0}_{j0}")
            nc.gpsimd.partition_broadcast(rb[:], rec[:], 128)
            nc.vector.tensor_mul(OT[h0][:, j0*CH:(j0+1)*CH],
                                 po0[:], rb[:])

        def emit_C(j, thunks=()):
            nk = 4 * (j + 1)
            thunks = list(thunks)
            for h in range(HL):
                po = psO.tile([128, CH], F32, tag="po", name=f"po{h}_{j}")
                acc_a = accA.tile([128, CH], BF16, tag="aa",
                                  name=f"aa{h}_{j}")
                acc_b = accB.tile([128, CH], BF16, tag="ab",
                                  name=f"ab{h}_{j}")
                qslice = qT[h][:, j*CH:(j+1)*CH]
                for t in range(nk):
                    i = t - 4*j
                    lo = 128*i if i > 0 else 0
                    # j=0/t=1 runs full width so the acc_b accumulator is
                    # fully written (exact zeros in the masked columns)
                    el = 0 if (j == 0 and t == 1) else lo
                    ps = psS.tile([128, CH], F32, tag="ps",
                                  name=f"ps{h}_{j}_{t}")
                    nc.tensor.matmul(
                        ps[:, el:CH], kTt[h][:, t*128:(t+1)*128],
                        qslice[:, el:CH], start=True, stop=True)
                    if i >= 0:
                        # only up to the diagonal block needs masking
                        nc.vector.tensor_add(
                            ps[:, el:lo+128], ps[:, el:lo+128],
                            mk[i][:, el:lo+128])
                    if t == 0:
                        nc.scalar.activation(acc_a[:, lo:CH],
                                             ps[:, lo:CH], EXP)
                        rhs = acc_a[:, lo:CH]
                    elif t == 1:
                        nc.scalar.activation(acc_b[:, el:CH],
                                             ps[:, el:CH], EXP)
                        rhs = acc_b[:, lo:CH]
                    else:
                        ex = stageP.tile([128, CH], BF16, tag="stg",
                                         name=f"ex{h}{j}{t}")
                        nc.scalar.activation(ex[:, lo:CH], ps[:, lo:CH],
                                             EXP)
                        if t % 2 == 0:
                            nc.vector.tensor_add(
                                acc_a[:, lo:CH], acc_a[:, lo:CH],
                                ex[:, lo:CH])
                        else:
                            nc.gpsimd.tensor_add(
                                acc_b[:, lo:CH], acc_b[:, lo:CH],
                                ex[:, lo:CH])
                        rhs = ex[:, lo:CH]
                    nc.tensor.matmul(
                        po[:, lo:CH], V[t][:, h*128:(h+1)*128], rhs,
                        start=(t == 0), stop=(t == nk - 1),
                        skip_group_check=True)
                pending.append((h, j, po, acc_a, acc_b))
                if len(pending) > 1:
                    finish_one()
                for _ in range(6):
                    if thunks:
                        thunks.pop(0)()
            return thunks

        def emit_D(j):
            for m in range(4*j, 4*j + 4):
                for n in range(8):
                    pf = psF.tile([128, 512], F32, tag="pf",
                                  name=f"pf{m}_{n}")
                    for h in range(HL):
                        nc.tensor.matmul(
                            pf[:], OT[h][:, m*128:(m+1)*128],
                            wo_sb[:, h*D + n*512: h*D + (n+1)*512],
                            start=(h == 0), stop=(h == HL - 1))
                    ob = outD.tile([128, 512], BF16, tag="ob",
                                   name=f"ob{m}_{n}")
                    if (m * 8 + n) % 2 == 0:
                        nc.vector.tensor_copy(ob[:], pf[:])
                    else:
                        nc.scalar.copy(ob[:], pf[:])
                    nc.sync.dma_start(
                        out[m*128:(m+1)*128, n*512:(n+1)*512], ob[:])

        emit_C(0)
        left = emit_C(1, rope_q4)
        for th in left:
            th()
        emit_D(0)
        emit_C(2)
        emit_D(1)
        emit_C(3)
        emit_D(2)
        while pending:
            finish_one()
        emit_D(3)

    ropeA.release()
    trigA.release()
    miscC.release()
    accB.release()
    accA.release()
    stRotP.release()
    stageP.release()
    pv_pool.release()
    pqk.release()
    pot.release()
    small.release()


def prep_in_maps(hidden_states, attention_mask, cos, sin, Wq, Wk, Wv, Wo):
    bf16 = ml_dtypes.bfloat16
    hs = np.ascontiguousarray(np.asarray(hidden_states)[0], dtype=np.float32)
    mask = np.asarray(attention_mask, np.float32)[0, 0]
    cosT = np.ascontiguousarray(np.asarray(cos, np.float32)[0, 0].T)
    sinT = np.ascontiguousarray(np.asarray(sin, np.float32)[0, 0].T)
    sinTm = np.concatenate([-sinT[:64], sinT[64:]], 0)
    sc = np.float32(1.0 / math.sqrt(HD))
    hiddenT = np.ascontiguousarray(hs.T.astype(bf16))
    maskTd = np.stack([np.ascontiguousarray(mask[0:CH, t*128:(t+1)*128].T)
                       for t in range(4)], 0)

    shared = dict(hT=hiddenT, cosT=cosT.astype(bf16),
                  sinTm=np.ascontiguousarray(sinTm).astype(bf16),
                  maskT=np.ascontiguousarray(maskTd).astype(bf16))
    in_maps = []
    for c in range(NCORES):
        rows = slice(c * 512, (c + 1) * 512)
        in_maps.append(dict(
            shared,
            wqT=np.ascontiguousarray(
                np.asarray(Wq, np.float32)[rows].T.astype(bf16)),
            wkT=np.ascontiguousarray(
                (np.asarray(Wk, np.float32)[rows].T * sc).astype(bf16)),
            wvT=np.ascontiguousarray(
                np.asarray(Wv, np.float32)[rows].T.astype(bf16)),
            woT=np.ascontiguousarray(
                np.asarray(Wo, np.float32)[:, rows].T.astype(bf16)),
        ))
    return in_maps


_NC_CACHE = {}


def get_nc():
    if "nc" not in _NC_CACHE:
        _NC_CACHE["nc"] = build_nc()
    return _NC_CACHE["nc"]


def kernel(hidden_states, attention_mask, cos, sin, Wq, Wk, Wv, Wo,
           **run_kwargs):
    in_maps = prep_in_maps(hidden_states, attention_mask, cos, sin,
                           Wq, Wk, Wv, Wo)
    nc = get_nc()
    res = run_bass_kernel_spmd(nc, in_maps, core_ids=list(range(NCORES)),
                               **run_kwargs)
    total = np.zeros((S, D), dtype=np.float32)
    for r in res.results:
        total += np.asarray(r["out"], dtype=np.float32)
    out = total[None]  # [1, S, D]
    _NC_CACHE["last_results"] = res
    return out

